# revision 1
# baseline (speedup 1.0000x reference)
"""Trainium2 Bass kernel for nn_ClassifierGuided (2-modality top-12-of-16 MoE classifier).

Sharding: pure data-parallel over tokens. 2 modalities x 4096 tokens = 8192
tokens; each of the 8 cores owns 1024 tokens of one modality (cores 0-3 ->
modality 0, cores 4-7 -> modality 1) and that modality's full weights.
Dense-eval MoE (all 16 experts computed, sparse gates applied), so no
all-to-all is needed.

Per-core math (transposed activation layout, d on partitions):
  gating   : logits = x @ Wg -> top-12 mask -> softmax -> gates g [B,16]
  experts  : h_e = relu(x @ W1_e + b1_e); hg_e = g_e * h_e
  combine  : moeT = sum_e W2_e^T @ hg_e  (+ b2^T @ g^T folded as one matmul)
  residual : z = relu(moe) + x
  head     : outT = Wo^T @ zT + bo

All matmuls run as float32r (full-rate fp32 PE path, ~1e-4 relative rounding).
Experts are processed in pairs so every expert matmul is a full 128x128 tile
(192+192 = 384 = 3*128 h-dims per pair).
"""
import sys

sys.path.insert(0, "/opt/trn_rl_repo")

import numpy as np

import concourse.bass as bass
import concourse.mybir as mybir
import concourse.tile as tile
from concourse import bacc
from concourse.bass_utils import run_bass_kernel_spmd
from concourse.masks import make_identity

# ---- problem sizes (hardcoded per the harness contract) ----
B = 4096           # tokens per modality
D = 768            # model dim
E = 16             # experts
H = 192            # expert hidden
O = 101            # classifier out
KTOP = 12          # top-k experts
NCORES = 8
BC = B // 4        # 1024 tokens per core
DC = D // 128      # 6 d-chunks
NT = 512           # token tile (matmul moving dim)
NTILES = BC // NT  # 2
NPAIR = E // 2     # 8 expert pairs
HP = 2 * H         # 384 h-dims per pair = 3 chunks of 128
HCH = HP // 128    # 3
F32 = mybir.dt.float32
F32R = mybir.dt.float32r
NEG_BIG = -1.0e30

_NC_CACHE = {}
DEBUG = False


def build_nc():
    nc = bacc.Bacc("TRN2", target_bir_lowering=False, debug=False,
                   num_devices=NCORES)

    # ---- DRAM I/O (per-core views; host pre-packs) ----
    xT = nc.dram_tensor("xT", [D, BC], F32R, kind="ExternalInput").ap()
    w1p = nc.dram_tensor("w1p", [D, E * H], F32R, kind="ExternalInput").ap()
    w2p = nc.dram_tensor("w2p", [E * H, D], F32R, kind="ExternalInput").ap()
    b1p = nc.dram_tensor("b1p", [128, E * H // 128], F32, kind="ExternalInput").ap()
    b2 = nc.dram_tensor("b2", [E, D], F32R, kind="ExternalInput").ap()
    wg = nc.dram_tensor("wg", [D, E], F32R, kind="ExternalInput").ap()
    wo = nc.dram_tensor("wo", [D, O], F32R, kind="ExternalInput").ap()
    bo = nc.dram_tensor("bo", [O, 1], F32, kind="ExternalInput").ap()
    outT = nc.dram_tensor("outT", [O, BC], F32, kind="ExternalOutput").ap()
    if DEBUG:
        dbg_gT = nc.dram_tensor("dbg_gT", [E, BC], F32, kind="ExternalOutput").ap()
        dbg_gb = nc.dram_tensor("dbg_gb", [128, 2, NT], F32, kind="ExternalOutput").ap()
        dbg_hg = nc.dram_tensor("dbg_hg", [128, NT], F32, kind="ExternalOutput").ap()
        dbg_h = nc.dram_tensor("dbg_h", [128, NT], F32, kind="ExternalOutput").ap()
        dbg_moe = nc.dram_tensor("dbg_moe", [128, DC, NT], F32, kind="ExternalOutput").ap()

    with tile.TileContext(nc) as tc:
        with tc.tile_pool(name="const", bufs=1) as cpool:
            # resident SBUF tensors
            xsb = cpool.tile([128, DC, BC], F32R)          # xT; later holds zT
            w1sb = cpool.tile([128, DC, E * H], F32R)
            b1sb = cpool.tile([128, E * H // 128], F32)
            b2sb = cpool.tile([E, D], F32R)
            wosb = cpool.tile([128, DC, O], F32R)
            bosb = cpool.tile([O, 1], F32)
            gT = cpool.tile([E, BC], F32R)                 # gates, expert-major
            wgf = cpool.tile([128, DC, E], F32)            # fp32 gating weights
            ident = cpool.tile([128, 128], F32)

            make_identity(nc, ident[:, :])

            # loads ordered by first use: wg + x (gating) split across the SP
            # and ACT HWDGE queues, then W1 by pair interleaved with the small
            # tensors so pair 0 lands as early as possible
            nc.sync.dma_start(out=wgf[:, :, :],
                              in_=wg.bitcast(F32).rearrange("(c p) e -> p c e", p=128))
            for c in range(DC):
                eng = nc.sync if c < 3 else nc.scalar
                eng.dma_start(out=xsb[:, c, :], in_=xT[128 * c:128 * (c + 1), :])
            w1v = w1p.rearrange("(c q) h -> q c h", q=128)

            def load_w1(p):
                nc.sync.dma_start(out=w1sb[:, :, HP * p:HP * (p + 1)],
                                  in_=w1v[:, :, HP * p:HP * (p + 1)])

            w2tiles = {}
            ctx_w2 = tc.tile_pool(name="w2pool", bufs=3)
            w2pool = ctx_w2.__enter__()

            def load_w2(t, p):
                # W2 on the SP queue (not ACT: transfers there block the
                # latency-critical relu chain); one DMA per pair
                w2 = w2pool.tile([128, HCH, D], F32R, tag="w2", name="w2t")
                nc.sync.dma_start(
                    out=w2[:, :, :],
                    in_=w2p[HP * p:HP * (p + 1), :].rearrange(
                        "(m q) d -> q m d", q=128))
                w2tiles[(t, p)] = w2

            load_w1(0)
            nc.sync.dma_start(out=b1sb[:, :], in_=b1p[:, :])
            load_w1(1)
            load_w2(0, 0)
            load_w1(2)
            load_w2(0, 1)
            nc.sync.dma_start(out=b2sb[:, :], in_=b2[:, :])
            load_w1(3)
            load_w2(0, 2)
            for c in range(DC):
                nc.sync.dma_start(out=wosb[:, c, :], in_=wo[128 * c:128 * (c + 1), :])
            nc.sync.dma_start(out=bosb[:, :], in_=bo[:, :])
            for p in range(4, NPAIR):
                load_w1(p)
                load_w2(0, p - 1)
            load_w2(0, NPAIR - 1)

            # gates round-trip through DRAM; gate-broadcast tiles are read
            # back with a partition-step-0 DMA (POOL partition_broadcast is
            # HW-limited to out-base-partition 0)
            gdram = cpool.tile([E, BC], F32R, space="DRAM")
            gdram_ap = gdram
            gb_pre = {}
            gbpool = ctx_gb = tc.tile_pool(name="gbpool", bufs=2)
            gbpool = ctx_gb.__enter__()

            def load_gb(t, p):
                # gb[:,0,:] = gate(e0) broadcast over partitions; [:,1,:] = e1
                gb = gbpool.tile([128, 2, NT], F32R, tag="gb", name="gb")
                gb_src = bass.AP(tensor=gdram.tensor,
                                 offset=2 * p * BC + NT * t,
                                 ap=[[0, 128], [BC, 2], [1, NT]])
                nc.gpsimd.dma_start(out=gb[:, :, :], in_=gb_src)
                return gb

            # ---------------- gating pass (128-token subtiles) ----------------
            with tc.tile_pool(name="gps", bufs=2, space="PSUM") as gps, \
                 tc.tile_pool(name="gtp", bufs=2, space="PSUM") as gtp, \
                 tc.tile_pool(name="gsb", bufs=3) as gsb, \
                 tc.tile_pool(name="xgpool", bufs=2) as xgpool:
                xTv32 = xT.bitcast(F32).rearrange("(c q) b -> q c b", q=128)
                for i in range(BC // 128):
                    if i * 128 % NT == 0 and i > 0:
                        # flush finished token-half of the gates to DRAM early
                        lo = i * 128 - NT
                        nc.gpsimd.dma_start(out=gdram_ap[:, lo:lo + NT],
                                            in_=gT[:, lo:lo + NT])
                        if lo == 0:
                            gb_pre[(0, 0)] = load_gb(0, 0)
                            gb_pre[(0, 1)] = load_gb(0, 1)
                    ts = slice(128 * i, 128 * (i + 1))
                    # fp32-typed copies so the logits matmul runs in exact fp32
                    # (top-12 selection then matches the fp32 reference)
                    xg = xgpool.tile([128, DC, 128], F32, tag="xg", name="xg")
                    nc.gpsimd.dma_start(out=xg[:, :, :], in_=xTv32[:, :, ts])
                    lg_ps = gps.tile([128, E], F32, tag="lg")
                    for c in range(DC):
                        nc.tensor.matmul(lg_ps[:, :], xg[:, c, :], wgf[:, c, :],
                                         start=(c == 0), stop=(c == DC - 1))
                    lg = gsb.tile([128, E], F32, tag="lg_sb")
                    nc.vector.tensor_copy(lg[:, :], lg_ps[:, :])
                    # top-8 values, then values 9..16 after masking them out
                    t8a = gsb.tile([128, 8], F32, tag="t8a")
                    nc.vector.max(t8a[:, :], lg[:, :])
                    l2 = gsb.tile([128, E], F32, tag="l2")
                    nc.vector.match_replace(l2[:, :], t8a[:, :], lg[:, :], NEG_BIG)
                    t8b = gsb.tile([128, 8], F32, tag="t8b")
                    nc.vector.max(t8b[:, :], l2[:, :])
                    # softmax over entries >= 12th-largest (t8b[:,3])
                    e16 = gsb.tile([128, E], F32, tag="e16")
                    nc.scalar.activation(e16[:, :], lg[:, :],
                                         mybir.ActivationFunctionType.Exp)
                    em = gsb.tile([128, E], F32, tag="em")
                    ssum = gsb.tile([128, 1], F32, tag="ssum")
                    nc.vector.scalar_tensor_tensor(
                        out=em[:, :], in0=lg[:, :], scalar=t8b[:, 3:4],
                        in1=e16[:, :], op0=mybir.AluOpType.is_ge,
                        op1=mybir.AluOpType.mult, accum_out=ssum[:, :])
                    rinv = gsb.tile([128, 1], F32, tag="rinv")
                    nc.vector.reciprocal(rinv[:, :], ssum[:, :])
                    g = gsb.tile([128, E], F32, tag="g")
                    nc.vector.tensor_scalar_mul(g[:, :], em[:, :], rinv[:, :])
                    # transpose to expert-major gT[16, tokens]
                    gt_ps = gtp.tile([E, 128], F32, tag="gt")
                    nc.tensor.transpose(gt_ps[:, :], g[:, :], ident[:, :])
                    nc.vector.tensor_copy(gT[:, ts], gt_ps[:, :])
            if DEBUG:
                nc.sync.dma_start(out=dbg_gT[:, :], in_=gT[:, :].bitcast(F32))

            nc.gpsimd.dma_start(out=gdram_ap[:, BC - NT:], in_=gT[:, BC - NT:])

            # ---------------- main loop ----------------
            with tc.tile_pool(name="moeps", bufs=DC, space="PSUM") as moeps, \
                 tc.tile_pool(name="hps", bufs=2, space="PSUM") as hps, \
                 tc.tile_pool(name="gstpool", bufs=2) as gstpool, \
                 tc.tile_pool(name="hgpool", bufs=(20 if DEBUG else 22)) as hgpool, \
                 tc.tile_pool(name="opool", bufs=2) as opool:
                for t in range(NTILES):
                    ts = slice(NT * t, NT * (t + 1))
                    # one PSUM tile per d-chunk: a single big tile would put a
                    # false tile-level WAR between chunk c's drain (DVE read)
                    # and chunk c+1's accumulation (PE write)
                    moe = [moeps.tile([128, NT], F32, tag="moe", name="moe")
                           for _ in range(DC)]
                    w2t = [None] * NPAIR
                    hg = [[None] * HCH for _ in range(NPAIR)]

                    def stage1(p, ts=ts, moe=moe, w2t=w2t, hg=hg, t=t):
                        w2t[p] = w2tiles.pop((t, p), None)
                        if w2t[p] is None:
                            load_w2(t, p)
                            w2t[p] = w2tiles.pop((t, p))
                        gb = gb_pre.pop((t, p), None)
                        if gb is None:
                            gb = load_gb(t, p)
                        if DEBUG and t == 0 and p == 0:
                            nc.sync.dma_start(out=dbg_gb[:, :, :], in_=gb[:, :, :].bitcast(F32))
                        for m in range(HCH):
                            hcol = HP * p + 128 * m
                            hps_t = hps.tile([128, NT], F32, tag="h")
                            for c in range(DC):
                                nc.tensor.matmul(hps_t[:, :],
                                                 w1sb[:, c, hcol:hcol + 128],
                                                 xsb[:, c, ts],
                                                 start=(c == 0), stop=(c == DC - 1))
                            # relu(u + b1) in-place in PSUM, then gate-multiply to SBUF
                            nc.scalar.activation(hps_t[:, :], hps_t[:, :],
                                                 mybir.ActivationFunctionType.Relu,
                                                 bias=b1sb[:, hcol // 128:hcol // 128 + 1])
                            if DEBUG and t == 0 and p == 0 and m == 0:
                                dbg_h_sb = gstpool.tile([128, NT], F32, tag="dbg", name="dbg_h_sb")
                                nc.vector.tensor_copy(dbg_h_sb[:, :], hps_t[:, :])
                                nc.sync.dma_start(out=dbg_h[:, :], in_=dbg_h_sb[:, :])
                            hg[p][m] = hgpool.tile([128, NT], F32R, tag="hg", name="hg")
                            if m == 1:
                                # mixed chunk: parts 0:64 are e0's h[128:192],
                                # parts 64:128 are e1's h[0:64]
                                nc.vector.tensor_tensor(
                                    out=hg[p][m][0:64, :], in0=hps_t[0:64, :],
                                    in1=gb[0:64, 0, :].bitcast(F32),
                                    op=mybir.AluOpType.mult)
                                nc.vector.tensor_tensor(
                                    out=hg[p][m][64:128, :], in0=hps_t[64:128, :],
                                    in1=gb[64:128, 1, :].bitcast(F32),
                                    op=mybir.AluOpType.mult)
                            else:
                                nc.vector.tensor_tensor(
                                    out=hg[p][m][:, :], in0=hps_t[:, :],
                                    in1=gb[:, 0 if m == 0 else 1, :].bitcast(F32),
                                    op=mybir.AluOpType.mult)
                            if DEBUG and t == 0 and p == 0 and m == 0:
                                nc.sync.dma_start(out=dbg_hg[:, :], in_=hg[p][m][:, :].bitcast(F32))

                    def stage2(p, moe=moe, w2t=w2t, hg=hg, ts=ts, close=False):
                        if not close:
                            # m-outer: the first 12 matmuls need only hg m0/m1,
                            # giving hg m2's relu+mult chain extra cover
                            for m in range(HCH):
                                for c in range(DC):
                                    nc.tensor.matmul(moe[c][:, :],
                                                     w2t[p][:, m, 128 * c:128 * (c + 1)],
                                                     hg[p][m][:, :],
                                                     start=(p == 0 and m == 0), stop=False)
                            return
                        for c in range(DC):
                            for m in range(HCH):
                                nc.tensor.matmul(moe[c][:, :],
                                                 w2t[p][:, m, 128 * c:128 * (c + 1)],
                                                 hg[p][m][:, :],
                                                 start=(p == 0 and m == 0), stop=False)
                            if close:
                                # b2 bias term closes this chunk's accumulation
                                nc.tensor.matmul(moe[c][:, :],
                                                 b2sb[:, 128 * c:128 * (c + 1)],
                                                 gT[:, ts], start=False, stop=True)
                                finish_chunk(c)
                                # head matmul trails two chunks behind so its
                                # relu+residual drain is already complete
                                if c >= 2:
                                    head_chunk(c - 2)
                        if close:
                            head_chunk(DC - 2)
                            head_chunk(DC - 1)

                    def finish_chunk(c, moe=moe, ts=ts):
                        # z = relu(moe) + x in one DVE op, overwriting x in place
                        if DEBUG and t == 0:
                            dbg_moe_sb = gstpool.tile([128, NT], F32, tag="dbg", name="dbg_moe_sb")
                            nc.vector.tensor_copy(dbg_moe_sb[:, :], moe[c][:, :])
                            nc.sync.dma_start(out=dbg_moe[:, c, :], in_=dbg_moe_sb[:, :])
                        nc.vector.scalar_tensor_tensor(
                            out=xsb[:, c, ts], in0=moe[c][:, :], scalar=0.0,
                            in1=xsb[:, c, ts].bitcast(F32),
                            op0=mybir.AluOpType.max, op1=mybir.AluOpType.add)

                    out_ps_box = [None]

                    def head_chunk(c, ts=ts):
                        if out_ps_box[0] is None:
                            out_ps_box[0] = hps.tile([O, NT], F32, tag="h",
                                                     name="out_ps")
                        nc.tensor.matmul(out_ps_box[0][:, :], wosb[:, c, :],
                                         xsb[:, c, ts],
                                         start=(c == 0), stop=(c == DC - 1))

                    # software pipeline: stage1(p+1) covers stage2(p) latency;
                    # the last pair closes each moe chunk so relu/residual/head
                    # drain per chunk while later chunks still accumulate
                    stage1(0)
                    for p in range(NPAIR):
                        if p + 1 < NPAIR:
                            stage1(p + 1)
                        stage2(p, close=(p == NPAIR - 1))
                    out_ps = out_ps_box[0]
                    osb = opool.tile([O, NT], F32, tag="osb")
                    nc.scalar.activation(osb[:, :], out_ps[:, :],
                                         mybir.ActivationFunctionType.Identity,
                                         bias=bosb[:, :])
                    nc.sync.dma_start(out=outT[:, ts], in_=osb[:, :])
            ctx_gb.__exit__(None, None, None)
            ctx_w2.__exit__(None, None, None)

    nc.compile()
    return nc


def _pack_core_inputs(x, Wg, W1, b1, W2, b2, Wo, bo, c4):
    """Per-core input dict for one modality's weights + 1024-token slice."""
    f = np.float32
    tok = slice(BC * c4, BC * (c4 + 1))
    return {
        "xT": np.ascontiguousarray(np.asarray(x[tok], f).T),
        "w1p": np.ascontiguousarray(np.asarray(W1, f).transpose(1, 0, 2).reshape(D, E * H)),
        "w2p": np.ascontiguousarray(np.asarray(W2, f).reshape(E * H, D)),
        "b1p": np.ascontiguousarray(np.asarray(b1, f).reshape(-1).reshape(E * H // 128, 128).T),
        "b2": np.ascontiguousarray(np.asarray(b2, f)),
        "wg": np.ascontiguousarray(np.asarray(Wg, f)),
        "wo": np.ascontiguousarray(np.asarray(Wo, f)),
        "bo": np.ascontiguousarray(np.asarray(bo, f).reshape(O, 1)),
    }


def run_on_hw(inputs, trace=False, **kw):
    if "nc" not in _NC_CACHE:
        _NC_CACHE["nc"] = build_nc()
    nc = _NC_CACHE["nc"]
    in_maps = []
    for core in range(NCORES):
        i, c4 = divmod(core, 4)
        x = inputs["x0"] if i == 0 else inputs["x1"]
        in_maps.append(_pack_core_inputs(
            x, inputs["Wg"][i], inputs["W1"][i], inputs["b1"][i],
            inputs["W2"][i], inputs["b2"][i], inputs["Wo"][i], inputs["bo"][i], c4))
    res = run_bass_kernel_spmd(nc, in_maps, core_ids=list(range(NCORES)),
                               trace=trace, **kw)
    outs = []
    for i in range(2):
        outs.append(np.concatenate(
            [res.results[4 * i + c]["outT"].T for c in range(4)], axis=0))
    return (outs[0], outs[1]), res


def kernel(**inputs):
    (o0, o1), _ = run_on_hw(inputs)
    return (o0, o1)



# revision 8
# speedup vs baseline: 1.5940x; 1.5940x over previous
"""Trainium2 Bass kernel for nn_ClassifierGuided (2-modality top-12-of-16 MoE classifier).

Sharding: pure data-parallel over tokens. 2 modalities x 4096 tokens = 8192
tokens; each of the 8 cores owns 1024 tokens of one modality (cores 0-3 ->
modality 0, cores 4-7 -> modality 1) and that modality's full weights.
Dense-eval MoE (all 16 experts computed, sparse gates applied), so no
all-to-all is needed.

Core math (per core, transposed activation layout, d on partitions):
  gating   : lgn = -x @ Wg (exact fp32) -> top-12 via max8 of negated logits
             -> masked softmax -> gates gT [16, B] (stored as g/32, bf16)
  experts  : FC1/FC2 run in fp8 e4m3 with DoubleRow perf mode (256-deep
             contraction, 0.5 PE cycles/row = 4x fp32r throughput):
               psum1 = (16x)_fp8 @ (64 W1)_fp8         = 1024 u
               h     = relu(psum1 + 1024 b1)           = 1024 h   (bf16)
               hq    = h * broadcast(g/32)              = 32 g h   (fp8)
               psum2 = hq @ (64 W2)_fp8 + (65536 b2) @ (g/32) = 2048 moe
  residual : z' = relu(psum2) + 2048x (single DVE op; x pre-scaled on host)
  head     : out = (Wo/2048)^T @ z' + bo  (fp32r, exact scale cancellation)

All scale factors are powers of two, so gating logits are bit-identical to an
unscaled fp32 evaluation and the head matmul sees an exactly rescaled z.
"""
import sys

sys.path.insert(0, "/opt/trn_rl_repo")

import ml_dtypes
import numpy as np

import concourse.bass as bass
import concourse.mybir as mybir
import concourse.tile as tile
from concourse import bacc
from concourse.bass_utils import run_bass_kernel_spmd
from concourse.masks import make_identity

# ---- problem sizes (hardcoded per the harness contract) ----
B = 4096           # tokens per modality
D = 768            # model dim
E = 16             # experts
H = 192            # expert hidden
O = 101            # classifier out
NCORES = 8
BC = B // 4        # 1024 tokens per core
DC = D // 128      # 6 d-chunks
NT = 512           # token tile (matmul moving dim)
NTILES = BC // NT  # 2
NPAIR = E // 2     # 8 expert pairs
NQUAD = E // 4     # 4 expert quads
HP = 2 * H         # 384 h-dims per pair
HCH = HP // 128    # 3 h-chunks per pair
F32 = mybir.dt.float32
F32R = mybir.dt.float32r
BF16 = mybir.dt.bfloat16
F8 = mybir.dt.float8e4
F8NP = ml_dtypes.float8_e4m3
BF16NP = ml_dtypes.bfloat16
DR = mybir.MatmulPerfMode.DoubleRow

_NC_CACHE = {}


def build_nc():
    nc = bacc.Bacc("TRN2", target_bir_lowering=False, debug=False,
                   num_devices=NCORES)

    # ---- DRAM I/O (per-core views; host pre-packs + pre-quantizes) ----
    xT = nc.dram_tensor("xT", [D, BC], F32R, kind="ExternalInput").ap()
    x8d = nc.dram_tensor("x8", [128, DC, BC], F8, kind="ExternalInput").ap()
    w1d = nc.dram_tensor("w1q", [128, NPAIR, DC, HP], F8,
                         kind="ExternalInput").ap()
    b1d = nc.dram_tensor("b1p", [128, E * H // 128], F32,
                         kind="ExternalInput").ap()
    w2d = nc.dram_tensor("w2q", [128, NQUAD, HCH, 2, D], F8,
                         kind="ExternalInput").ap()
    b2d = nc.dram_tensor("b2s", [E, D], BF16, kind="ExternalInput").ap()
    wgd = nc.dram_tensor("wgs", [D, E], F32, kind="ExternalInput").ap()
    wod = nc.dram_tensor("wos", [D, O], F32R, kind="ExternalInput").ap()
    bod = nc.dram_tensor("bo", [O, 1], F32, kind="ExternalInput").ap()
    outT = nc.dram_tensor("outT", [O, BC], F32, kind="ExternalOutput").ap()

    with tile.TileContext(nc) as tc:
        with tc.tile_pool(name="const", bufs=1) as cpool:
            # resident SBUF tensors
            xsb = cpool.tile([128, DC, BC], F32R)        # 2048 x^T
            x8sb = cpool.tile([128, DC, BC], F8)         # fp8(16 x^T)
            w1sb = cpool.tile([128, NPAIR, DC, HP], F8)
            w2sb = cpool.tile([128, NQUAD, HCH, 2, D], F8)
            b1sb = cpool.tile([128, E * H // 128], F32)  # 1024 b1
            b2sb = cpool.tile([E, D], BF16)              # 65536 b2
            wgf = cpool.tile([128, DC, E], F32)          # -Wg/2048
            wosb = cpool.tile([128, DC, O], F32R)        # Wo/2048
            bosb = cpool.tile([O, 1], F32)
            gT = cpool.tile([E, BC], BF16)               # gates g/32
            ident = cpool.tile([128, 128], F32)
            gdram = cpool.tile([E, BC], BF16, space="DRAM")

            make_identity(nc, ident[:, :])

            # ---- DMA schedule ----
            # SP queue: gating weights, fp8 x, then the fat fp32 resident x
            # (only needed from the first residual-add, ~mid-tile-0)
            nc.sync.dma_start(out=wgf[:, :, :],
                              in_=wgd.rearrange("(c p) e -> p c e", p=128))
            nc.sync.dma_start(out=w1sb[:, 0:4, :, :], in_=w1d[:, 0:4, :, :])
            nc.sync.dma_start(out=w1sb[:, 4:8, :, :], in_=w1d[:, 4:8, :, :])
            nc.sync.dma_start(out=w2sb[:, 2:4, :, :, :], in_=w2d[:, 2:4, :, :, :])
            nc.sync.dma_start(out=b2sb[:, :], in_=b2d)
            for c in range(DC):
                nc.sync.dma_start(out=xsb[:, c, :], in_=xT[128 * c:128 * (c + 1), :])
            nc.sync.dma_start(out=wosb[:, :, :],
                              in_=wod.rearrange("(c p) o -> p c o", p=128))
            nc.sync.dma_start(out=bosb[:, :], in_=bod)
            # ACT queue: fp8 x, b1, first FC2 half
            nc.scalar.dma_start(out=x8sb[:, :, :], in_=x8d)
            nc.scalar.dma_start(out=b1sb[:, :], in_=b1d)
            nc.scalar.dma_start(out=w2sb[:, 0:2, :, :, :], in_=w2d[:, 0:2, :, :, :])

            # gates round-trip through DRAM; gate-broadcast tiles are read
            # back with a partition-step-0 DMA on the pool queue
            gb_pre = {}
            ctx_gb = tc.tile_pool(name="gbpool", bufs=17)
            gbpool = ctx_gb.__enter__()

            def load_gb(t, p):
                gb = gbpool.tile([128, 2, NT], BF16, tag="gb", name="gb")
                gb_src = bass.AP(tensor=gdram.tensor,
                                 offset=2 * p * BC + NT * t,
                                 ap=[[0, 128], [BC, 2], [1, NT]])
                nc.gpsimd.dma_start(out=gb[:, :, :], in_=gb_src)
                gb_pre[(t, p)] = gb

            # ---------------- gating (interleaved with FC1 below) ----------
            # xg: fp32 copies of x^T per 128-token subtile so the logits
            # matmul is exact fp32 (top-12 selection matches the reference).
            # Subtiles 0/2 load on the SP queue, the rest on the pool queue,
            # so the first gating steps start as early as possible.
            xTv32 = xT.bitcast(F32).rearrange("(c q) b -> q c b", q=128)
            ctx_gx = tc.tile_pool(name="xgpool", bufs=8)
            xgpool = ctx_gx.__enter__()
            xg_tiles = {}
            for i in range(BC // 128):
                xg = xgpool.tile([128, DC, 128], F32, tag="xg", name="xg")
                eng = nc.sync if i in (0, 2) else nc.gpsimd
                eng.dma_start(out=xg[:, :, :],
                              in_=xTv32[:, :, 128 * i:128 * (i + 1)])
                xg_tiles[i] = xg

            ctx_gps = tc.tile_pool(name="gps", bufs=2, space="PSUM")
            gps = ctx_gps.__enter__()
            ctx_gsb = tc.tile_pool(name="gsb", bufs=4)
            gsb = ctx_gsb.__enter__()

            def gate_subtile(i):
                ts = slice(128 * i, 128 * (i + 1))
                xg = xg_tiles.pop(i)
                lgn = gps.tile([128, E], F32, tag="g", name="lgn")
                for c in range(DC):
                    nc.tensor.matmul(lgn[:, :], xg[:, c, :], wgf[:, c, :],
                                     start=(c == 0), stop=(c == DC - 1))
                # lgn holds -logits: max8 descending = logits ascending, so
                # entry 4 is -(12th-largest logit) = the top-12 threshold
                t8 = gsb.tile([128, 8], F32, tag="t8")
                nc.vector.max(t8[:, :], lgn[:, :])
                e16 = gsb.tile([128, E], F32, tag="e16")
                nc.scalar.activation(e16[:, :], lgn[:, :],
                                     mybir.ActivationFunctionType.Exp,
                                     scale=-1.0)
                em = gsb.tile([128, E], F32, tag="em")
                ssum = gsb.tile([128, 1], F32, tag="ssum")
                nc.vector.scalar_tensor_tensor(
                    out=em[:, :], in0=lgn[:, :], scalar=t8[:, 4:5],
                    in1=e16[:, :], op0=mybir.AluOpType.is_le,
                    op1=mybir.AluOpType.mult, accum_out=ssum[:, :])
                rinv = gsb.tile([128, 1], F32, tag="rinv")
                nc.vector.reciprocal(rinv[:, :], ssum[:, :])
                g = gsb.tile([128, E], F32, tag="g")
                nc.scalar.activation(g[:, :], em[:, :],
                                     mybir.ActivationFunctionType.Copy,
                                     scale=rinv[:, :])
                gt_ps = gps.tile([E, 128], F32, tag="g", name="gt_ps")
                nc.tensor.transpose(gt_ps[:, :], g[:, :], ident[:, :])
                # gates stored pre-scaled by 1/32 (fp8 hq carries the 32)
                nc.scalar.activation(gT[:, ts], gt_ps[:, :],
                                     mybir.ActivationFunctionType.Copy,
                                     scale=1.0 / 32.0)

            def flush_gates(t):
                lo = NT * t
                nc.gpsimd.dma_start(out=gdram[:, lo:lo + NT],
                                    in_=gT[:, lo:lo + NT])
                for p in range(NPAIR):
                    load_gb(t, p)

            # ---------------- main loop ----------------
            ctx_moe = tc.tile_pool(name="moeps", bufs=3, space="PSUM")
            moeps = ctx_moe.__enter__()
            ctx_hps = tc.tile_pool(name="hps", bufs=3, space="PSUM")
            hps = ctx_hps.__enter__()
            ctx_h32 = tc.tile_pool(name="h32pool", bufs=8)
            h32pool = ctx_h32.__enter__()
            ctx_hq = tc.tile_pool(name="hqpool", bufs=5)
            hqpool = ctx_hq.__enter__()
            ctx_op = tc.tile_pool(name="opool", bufs=2)
            opool = ctx_op.__enter__()

            def run_tile(t, next_cb=None):
                ts = slice(NT * t, NT * (t + 1))
                hq = [hqpool.tile([128, 2 * HCH, NT], F8, tag="hq", name="hq")
                      for _ in range(NQUAD)]
                moe = {}
                out_ps_box = [None]

                def stage1(p):
                    gb = gb_pre.pop((t, p))
                    q, base = p // 2, (p % 2) * HCH
                    for m in range(HCH):
                        hp = hps.tile([128, NT], F32, tag="h")
                        for j in range(DC // 2):
                            nc.tensor.matmul(
                                hp[:, :],
                                w1sb[:, p, 2 * j:2 * j + 2, 128 * m:128 * (m + 1)],
                                x8sb[:, 2 * j:2 * j + 2, ts],
                                start=(j == 0), stop=(j == DC // 2 - 1),
                                perf_mode=DR)
                        # h = relu(psum + 1024 b1), carried at 1024x scale
                        h32 = h32pool.tile([128, NT], BF16, tag="h32", name="h32")
                        nc.scalar.activation(
                            h32[:, :], hp[:, :],
                            mybir.ActivationFunctionType.Relu,
                            bias=b1sb[:, 3 * p + m:3 * p + m + 1])
                        dst = hq[q][:, base + m, :]
                        if m == 1:
                            # mixed chunk: parts 0:64 are e0's h[128:192],
                            # parts 64:128 are e1's h[0:64]
                            nc.gpsimd.tensor_tensor(
                                out=dst[0:64, :], in0=h32[0:64, :],
                                in1=gb[0:64, 0, :], op=mybir.AluOpType.mult)
                            nc.vector.tensor_tensor(
                                out=dst[64:128, :], in0=h32[64:128, :],
                                in1=gb[64:128, 1, :], op=mybir.AluOpType.mult)
                        else:
                            nc.gpsimd.tensor_tensor(
                                out=dst[:, :], in0=h32[:, :],
                                in1=gb[:, 0 if m == 0 else 1, :],
                                op=mybir.AluOpType.mult)

                def stage2(q, grp, close=False):
                    cs = range(3 * grp, 3 * grp + 3)
                    if grp == 0 and q == 0:
                        for c in cs:
                            moe[c] = moeps.tile([128, NT], F32, tag="moe",
                                                name="moe")
                    if grp == 1 and q == 0:
                        for c in cs:
                            moe[c] = moeps.tile([128, NT], F32, tag="moe",
                                                name="moe")
                    for r in range(HCH):
                        for c in cs:
                            nc.tensor.matmul(
                                moe[c][:, :],
                                w2sb[:, q, r, :, 128 * c:128 * (c + 1)],
                                hq[q][:, 2 * r:2 * r + 2, :],
                                start=(q == 0 and r == 0), stop=False,
                                perf_mode=DR)
                    if close:
                        for c in cs:
                            # b2 bias closes this chunk's accumulation
                            nc.tensor.matmul(moe[c][:, :],
                                             b2sb[:, 128 * c:128 * (c + 1)],
                                             gT[:, ts], start=False, stop=True)
                            # z' = relu(moe) + 2048x in one DVE op, in place
                            nc.vector.scalar_tensor_tensor(
                                out=xsb[:, c, ts], in0=moe[c][:, :], scalar=0.0,
                                in1=xsb[:, c, ts].bitcast(F32),
                                op0=mybir.AluOpType.max,
                                op1=mybir.AluOpType.add)

                def head_chunk(c):
                    if out_ps_box[0] is None:
                        out_ps_box[0] = hps.tile([O, NT], F32, tag="h",
                                                 name="out_ps")
                    nc.tensor.matmul(out_ps_box[0][:, :], wosb[:, c, :],
                                     xsb[:, c, ts],
                                     start=(c == 0), stop=(c == DC - 1))

                # software pipeline: FC1(pair) feeds FC2(quad) two pairs
                # behind; FC2 runs twice over d-groups (3 PSUM banks each)
                stage1(0)
                stage1(1)
                if t == 0:
                    gate_subtile(6)
                    gate_subtile(7)
                    flush_gates(1)
                stage1(2)
                stage1(3)
                stage2(0, 0)
                stage1(4)
                stage1(5)
                stage2(1, 0)
                stage1(6)
                stage1(7)
                stage2(2, 0)
                stage2(3, 0, close=True)
                stage2(0, 1)
                for c in range(3):
                    head_chunk(c)
                stage2(1, 1)
                stage2(2, 1)
                stage2(3, 1, close=True)
                if next_cb is not None:
                    next_cb()  # next tile's first FC1 pairs cover the drain
                for c in range(3, DC):
                    head_chunk(c)
                osb = opool.tile([O, NT], F32, tag="osb")
                nc.scalar.activation(osb[:, :], out_ps_box[0][:, :],
                                     mybir.ActivationFunctionType.Identity,
                                     bias=bosb[:, :])
                nc.sync.dma_start(out=outT[:, ts], in_=osb[:, :])

            # gating subtiles 0-5 interleave with tile-0 FC1 emission so no
            # engine queue is head-of-line blocked on late inputs
            gate_subtile(0)
            gate_subtile(1)
            gate_subtile(2)
            gate_subtile(3)
            flush_gates(0)
            gate_subtile(4)
            gate_subtile(5)

            run_tile(0)
            run_tile(1)

            for ctx in (ctx_op, ctx_hq, ctx_h32, ctx_hps, ctx_moe, ctx_gsb,
                        ctx_gps, ctx_gx, ctx_gb):
                ctx.__exit__(None, None, None)

    nc.compile()
    return nc


def _pack_core_inputs(x, Wg, W1, b1, W2, b2, Wo, bo, c4):
    """Per-core input dict for one modality's weights + 1024-token slice."""
    f = np.float32
    x = np.asarray(x, f)[BC * c4:BC * (c4 + 1)]            # [1024, 768]
    xt = np.ascontiguousarray(x.T)                         # [768, 1024]
    w1_2d = np.asarray(W1, f).transpose(1, 0, 2).reshape(D, E * H)
    w2_2d = np.asarray(W2, f).reshape(E * H, D)
    return {
        "xT": np.ascontiguousarray(2048.0 * xt),
        "x8": np.ascontiguousarray(
            (16.0 * xt).reshape(DC, 128, BC).transpose(1, 0, 2).astype(F8NP)),
        "w1q": np.ascontiguousarray(
            (64.0 * w1_2d).reshape(DC, 128, NPAIR, HP)
            .transpose(1, 2, 0, 3).astype(F8NP)),
        "b1p": np.ascontiguousarray(
            (1024.0 * np.asarray(b1, f)).reshape(E * H // 128, 128).T),
        "w2q": np.ascontiguousarray(
            (64.0 * w2_2d).reshape(NQUAD, HCH, 2, 128, D)
            .transpose(3, 0, 1, 2, 4).astype(F8NP)),
        "b2s": (65536.0 * np.asarray(b2, f)).astype(BF16NP),
        "wgs": np.ascontiguousarray(np.asarray(Wg, f) * (-1.0 / 2048.0)),
        "wos": np.ascontiguousarray(np.asarray(Wo, f) * (1.0 / 2048.0)),
        "bo": np.ascontiguousarray(np.asarray(bo, f).reshape(O, 1)),
    }


def run_on_hw(inputs, trace=False, **kw):
    if "nc" not in _NC_CACHE:
        _NC_CACHE["nc"] = build_nc()
    nc = _NC_CACHE["nc"]
    in_maps = []
    for core in range(NCORES):
        i, c4 = divmod(core, 4)
        x = inputs["x0"] if i == 0 else inputs["x1"]
        in_maps.append(_pack_core_inputs(
            x, inputs["Wg"][i], inputs["W1"][i], inputs["b1"][i],
            inputs["W2"][i], inputs["b2"][i], inputs["Wo"][i], inputs["bo"][i], c4))
    res = run_bass_kernel_spmd(nc, in_maps, core_ids=list(range(NCORES)),
                               trace=trace, **kw)
    outs = []
    for i in range(2):
        outs.append(np.concatenate(
            [res.results[4 * i + c]["outT"].T for c in range(4)], axis=0))
    return (outs[0], outs[1]), res


def kernel(**inputs):
    (o0, o1), _ = run_on_hw(inputs)
    return (o0, o1)
